# revision 21
# baseline (speedup 1.0000x reference)
"""Trainium2 Bass kernel for nn_MHA_36584531427723.

Sharding: 8 cores = 2 batches x 4 head-groups (4 heads of 64 dims each per
core). Each core computes its batch's Q/K/V projections restricted to its
head-group's 256 output features, attention for its 4 heads, and a partial
output projection (its 256 rows of Wo^T). The host sums the 4 partials per
batch and adds bo.

v3 design notes (engine balance + pipeline; cost-model driven):
  - Q/K projections run as fp8e4m3 DoubleRow matmuls (contraction 256/instr,
    0.5 cyc/row, psum output must start at partition 0 on hw): weights are
    host-prescaled by 64 (fp8 min-normal is 2^-6, raw weights sigma 0.02);
    the evict rescales by 1/64 and adds the bias. The V projection stays bf16:
    measured offline, fp8 V alone costs 1.8e-2 relative error (V errors pass
    straight to the output) while fp8 Q+K costs only 3.9e-3.
  - Activation runs ONLY the 256 exp instructions [128,1024] (~133us); all
    psum evicts go to DVE (GpSimd cannot touch PSUM on real hw).
  - Softmax denominator comes from a ones-column appended to V (psum row 64):
    reciprocal (DVE, crosses partitions 64->0), broadcast via a tiny f32r
    matmul into psum, then o = srp * o_raw writes oT at the head's partition
    base (DVE maps in/out partition ranges by index; verified on hw).
  - PV runs in two passes (qb 0,1 then qb 2,3) with kc as the OUTER loop and
    two live psum accumulators per pass. Each ex tile is [128,1024] (one
    q-half): the A-half ring is released during pass 1 and the B-half ring
    during pass 2, so head h+1's scores/exp stream while head h's PV runs
    with only ~32 ex tiles of SBUF.
  - Emission interleaving: pass1(h) emits B-half scores(h); pass2(h) emits
    A-half scores(h+1) (or the first 8 output-projection tiles when h=3).
    Phase A emits A-half scores of head 0 as K/V panels complete.
  - Mask multiply is split ~7/16 to GpSimd to balance DVE/Pool near 120us.
  - Biases for the Q/K evicts are host-tiled to [128,4] so the value is
    correct whether the scalar operand indexes by input or output lane.
  - PSUM: scores 2x[128,1024](4) + PV 2x[65,512](2) + srp(1) + outproj(1) = 8.
  - softmax max-subtraction and the +eps are dropped (|E|<~1 so exp is safe;
    relative effect ~1e-11).
"""

import numpy as np
import ml_dtypes

import concourse.bacc as bacc
import concourse.bass as bass  # noqa: F401
import concourse.mybir as mybir
import concourse.tile as tile
from concourse.bass_utils import run_bass_kernel_spmd

B, N, D = 2, 2048, 1024
H = 16
HD = 64
HL = 4  # heads per core
DL = HL * HD  # 256 local features
P = 128
KO = D // P  # 8 contraction chunks of the input feature dim
KO2 = KO // 2  # 4 DoubleRow chunk-pairs
NKC = N // P  # 16 k-token chunks
NPAN = 4
PANW = N // NPAN  # 512-wide token panels in the projection phase
SCALE = 1.0 / 32.0  # 1/sqrt(DIM_V)
WS = 64.0  # host-side weight prescale for fp8
INV_WS = 1.0 / WS

F32 = mybir.dt.float32
F32R = mybir.dt.float32r
BF16 = mybir.dt.bfloat16
FP8 = mybir.dt.float8e4
AF = mybir.ActivationFunctionType
ALU = mybir.AluOpType
DR = mybir.MatmulPerfMode.DoubleRow

NP_FP8 = ml_dtypes.float8_e4m3
NP_BF16 = ml_dtypes.bfloat16


def build_nc():
    nc = bacc.Bacc(None, target_bir_lowering=False)
    QT = nc.dram_tensor("qt", (D, N), FP8, kind="ExternalInput")
    KT = nc.dram_tensor("kt", (D, N), FP8, kind="ExternalInput")
    KTB = nc.dram_tensor("ktb", (D, N), BF16, kind="ExternalInput")
    MT = nc.dram_tensor("mt", (N, N), BF16, kind="ExternalInput")
    WQT = nc.dram_tensor("wqt", (D, DL), FP8, kind="ExternalInput")
    WKT = nc.dram_tensor("wkt", (D, DL), FP8, kind="ExternalInput")
    WVT = nc.dram_tensor("wvt", (D, DL), BF16, kind="ExternalInput")
    WOT = nc.dram_tensor("wot", (DL, D), BF16, kind="ExternalInput")
    BQT = nc.dram_tensor("bqt", (P, 4), F32, kind="ExternalInput")
    BKT = nc.dram_tensor("bkt", (P, 4), F32, kind="ExternalInput")
    BV = nc.dram_tensor("bv", (DL,), F32, kind="ExternalInput")
    ONES = nc.dram_tensor("ones", (HD,), F32R, kind="ExternalInput")
    OUT = nc.dram_tensor("out", (N, D), F32, kind="ExternalOutput")

    qt_r = QT[:].rearrange("(ko p) q -> p ko q", p=P)
    kt_r = KT[:].rearrange("(ko p) q -> p ko q", p=P)
    ktb_r = KTB[:].rearrange("(ko p) q -> p ko q", p=P)
    mt_r = MT[:].rearrange("(kc p) q -> p kc q", p=P)

    with tile.TileContext(nc) as tc:
        with (
            tc.tile_pool(name="persist", bufs=1) as persist,
            tc.tile_pool(name="expool", bufs=15) as expool,
            tc.tile_pool(name="sipool", bufs=2) as sipool,
            tc.tile_pool(name="osb", bufs=2) as osb,
            tc.tile_pool(name="spsum", bufs=2, space="PSUM") as spsum,
        ):
            # --- persistent tiles ---
            mT = persist.tile([P, NKC, N], BF16)  # 64KB/part
            qT = persist.tile([P, 2, N], BF16, tag="qT")
            kT = persist.tile([P, 2, N], BF16, tag="kT")
            oT = persist.tile([P, 2, N], BF16, tag="oT")
            v_sb = persist.tile([P, NKC, HL, HD + 1], BF16, tag="v")
            ones_sb = persist.tile([1, HD], F32R, tag="ones")
            bqt_sb = persist.tile([P, 4], F32, tag="bqt")
            bkt_sb = persist.tile([P, 4], F32, tag="bkt")
            bv_rep = persist.tile([P, DL], F32, tag="bv")
            wo_sb = persist.tile([P, 2, D], BF16, tag="wo")
            wq_sb = persist.tile([P, KO, DL], FP8, tag="wq")
            wk_sb = persist.tile([P, KO, DL], FP8, tag="wk")
            wv_sb = persist.tile([P, KO, DL], BF16, tag="wv")

            nc.vector.memset(v_sb[:, :, :, HD : HD + 1], 1.0)
            nc.sync.dma_start(out=ones_sb[:], in_=ONES[:][None])
            nc.sync.dma_start(out=bqt_sb[:], in_=BQT[:])
            nc.sync.dma_start(out=bkt_sb[:], in_=BKT[:])
            nc.sync.dma_start(out=bv_rep[:], in_=BV[:][None].to_broadcast((P, DL)))
            for w_sb, W in ((wq_sb, WQT), (wk_sb, WKT), (wv_sb, WVT)):
                nc.sync.dma_start(
                    out=w_sb[:], in_=W[:].rearrange("(ko p) m -> p ko m", p=P)
                )
            for cc in range(2):
                nc.sync.dma_start(
                    out=wo_sb[:, cc, :],
                    in_=WOT[:].rearrange("(cc p) n -> p cc n", p=P)[:, cc, :],
                )

            exA = [None] * NKC  # current A-half (q 0:1024) ex tiles, by kc
            exB = [None] * NKC  # current B-half (q 1024:2048)

            def scores_half(h, kc, half):
                """Scores+exp+mask for one [128 k, 1024 q] tile."""
                dc, po = h // 2, (h % 2) * HD
                ex = expool.tile(
                    [P, 1024],
                    BF16,
                    tag=("exA" if half == 0 else "exB"),
                    name=f"ex{h}_{kc}_{half}",
                )
                (exA if half == 0 else exB)[kc] = ex
                ps = spsum.tile([P, 1024], F32, tag="es", name=f"es{h}_{kc}_{half}")
                for j in range(2):
                    q0 = half * 1024 + j * 512
                    nc.tensor.matmul(
                        ps[:, j * 512 : (j + 1) * 512],
                        lhsT=kT[po : po + HD, dc, kc * P : (kc + 1) * P],
                        rhs=qT[po : po + HD, dc, q0 : q0 + 512],
                        start=True,
                        stop=True,
                    )
                nc.scalar.activation(out=ex[:], in_=ps[:], func=AF.Exp, scale=SCALE)
                eng = nc.gpsimd if (2 * kc + half) % 16 < 7 else nc.vector
                eng.tensor_mul(
                    out=ex[:],
                    in0=ex[:],
                    in1=mT[:, kc, half * 1024 : half * 1024 + 1024],
                )

            def pv_step(pso, h, qb, kc):
                exh = (exA if qb < 2 else exB)[kc]
                nc.tensor.matmul(
                    pso[:],
                    lhsT=v_sb[:, kc, h, :],
                    rhs=exh[:, (qb % 2) * 512 : (qb % 2) * 512 + 512],
                    start=(kc == 0),
                    stop=(kc == NKC - 1),
                )

            def normalize(pso, h, qb):
                dc, po = h // 2, (h % 2) * HD
                s_inv = sipool.tile([1, 512], F32R, tag="si", name=f"si{h}_{qb}")
                with nc.allow_low_precision(reason="f32r is bit-identical to f32"):
                    nc.vector.reciprocal(out=s_inv[:], in_=pso[HD : HD + 1, :])
                o_raw = sipool.tile([HD, 512], BF16, tag="oraw", name=f"or{h}_{qb}")
                nc.vector.tensor_copy(out=o_raw[:], in_=pso[0:HD, :])
                srp = srpsum.tile([HD, 512], F32, tag="srp", name=f"srp{h}_{qb}")
                nc.tensor.matmul(
                    srp[:], lhsT=ones_sb[:], rhs=s_inv[:], start=True, stop=True
                )
                q0 = qb * 512
                nc.vector.tensor_mul(
                    out=oT[po : po + HD, dc, q0 : q0 + 512], in0=srp[:], in1=o_raw[:]
                )

            def outproj(qt):
                # borrows a scores-ring psum tile (free of scores work by the
                # time outproj runs); single evict + single store per qt
                ps = spsum.tile([P, 1024], F32, tag="es", name=f"co{qt}")
                for nh in range(2):
                    for cc in range(2):
                        nc.tensor.matmul(
                            ps[:, nh * 512 : (nh + 1) * 512],
                            lhsT=oT[:, cc, qt * P : (qt + 1) * P],
                            rhs=wo_sb[:, cc, nh * 512 : (nh + 1) * 512],
                            start=(cc == 0),
                            stop=(cc == 1),
                        )
                o_sb = osb.tile([P, D], F32, tag="osb", name=f"osb{qt}")
                nc.vector.tensor_copy(out=o_sb[:], in_=ps[:])
                nc.scalar.dma_start(
                    out=OUT[qt * P : (qt + 1) * P, :], in_=o_sb[:]
                )

            # ---------------- Phase A: projections (+ h0 A-half scores) -----
            with (
                tc.tile_pool(name="panpool", bufs=2) as panpool,
                tc.tile_pool(name="panbpool", bufs=1) as panbpool,
                tc.tile_pool(name="pjpsum", bufs=2, space="PSUM") as pjpsum,
                tc.tile_pool(name="vpsum", bufs=2, space="PSUM") as vpsum,
            ):

                def q_panel(pan):
                    qs = slice(pan * PANW, (pan + 1) * PANW)
                    qt_pan = panpool.tile([P, KO, PANW], FP8, tag="pan", name=f"qp{pan}")
                    for ko in range(KO):
                        nc.sync.dma_start(out=qt_pan[:, ko, :], in_=qt_r[:, ko, qs])
                    for fc in range(4):
                        dc, po = fc // 2, (fc % 2) * HD
                        ps = pjpsum.tile([HD, PANW], F32, tag="pj", name=f"pjq{pan}_{fc}")
                        for ko2 in range(KO2):
                            nc.tensor.matmul(
                                ps[:],
                                lhsT=wq_sb[
                                    :, 2 * ko2 : 2 * ko2 + 2, fc * HD : (fc + 1) * HD
                                ],
                                rhs=qt_pan[:, 2 * ko2 : 2 * ko2 + 2, :],
                                start=(ko2 == 0),
                                stop=(ko2 == KO2 - 1),
                                perf_mode=DR,
                            )
                        nc.vector.tensor_scalar(
                            out=qT[po : po + HD, dc, qs],
                            in0=ps[:],
                            scalar1=INV_WS,
                            scalar2=bqt_sb[po : po + HD, fc : fc + 1],
                            op0=ALU.mult,
                            op1=ALU.add,
                        )

                def kv_panel(pan):
                    qs = slice(pan * PANW, (pan + 1) * PANW)
                    for kc in range(pan * 4, pan * 4 + 4):
                        nc.scalar.dma_start(out=mT[:, kc, :], in_=mt_r[:, kc, :])
                    kt_pan = panpool.tile([P, KO, PANW], FP8, tag="pan", name=f"kp{pan}")
                    ktb_pan = panbpool.tile(
                        [P, KO, PANW], BF16, tag="panb", name=f"kbp{pan}"
                    )
                    for ko in range(KO):
                        nc.sync.dma_start(out=kt_pan[:, ko, :], in_=kt_r[:, ko, qs])
                        nc.sync.dma_start(out=ktb_pan[:, ko, :], in_=ktb_r[:, ko, qs])
                    for fc in range(4):
                        dc, po = fc // 2, (fc % 2) * HD
                        ps = pjpsum.tile([HD, PANW], F32, tag="pj", name=f"pjk{pan}_{fc}")
                        for ko2 in range(KO2):
                            nc.tensor.matmul(
                                ps[:],
                                lhsT=wk_sb[
                                    :, 2 * ko2 : 2 * ko2 + 2, fc * HD : (fc + 1) * HD
                                ],
                                rhs=kt_pan[:, 2 * ko2 : 2 * ko2 + 2, :],
                                start=(ko2 == 0),
                                stop=(ko2 == KO2 - 1),
                                perf_mode=DR,
                            )
                        nc.vector.tensor_scalar(
                            out=kT[po : po + HD, dc, qs],
                            in0=ps[:],
                            scalar1=INV_WS,
                            scalar2=bkt_sb[po : po + HD, fc : fc + 1],
                            op0=ALU.mult,
                            op1=ALU.add,
                        )
                    # V projection in bf16 (fp8 V costs 1.8e-2 output error)
                    for tc128 in range(PANW // P):
                        kc = pan * 4 + tc128
                        psv = vpsum.tile(
                            [P, DL], F32, tag="pv8", name=f"pv8{pan}_{tc128}"
                        )
                        for ko in range(KO):
                            nc.tensor.matmul(
                                psv[:],
                                lhsT=ktb_pan[:, ko, tc128 * P : (tc128 + 1) * P],
                                rhs=wv_sb[:, ko, :],
                                start=(ko == 0),
                                stop=(ko == KO - 1),
                            )
                        nc.vector.tensor_add(
                            out=v_sb[:, kc, :, 0:HD],
                            in0=psv[:].rearrange("p (h d) -> p h d", h=HL),
                            in1=bv_rep[:].rearrange("p (h d) -> p h d", h=HL),
                        )
                    # head-0 A-half scores (need qT q 0:1024 = Q panels 0,1)
                    for kc in range(pan * 4, pan * 4 + 4):
                        scores_half(0, kc, 0)

                # A-half scores only need Q panels 0,1 -- start attention as
                # early as possible and fill the rest of Q between K/V panels
                q_panel(0)
                q_panel(1)
                kv_panel(0)
                kv_panel(1)
                q_panel(2)
                q_panel(3)
                kv_panel(2)
                kv_panel(3)

            # ---------------- Phase B+C: attention pipeline ----------------
            with (
                tc.tile_pool(name="pvpsum", bufs=3, space="PSUM") as pvpsum,
                tc.tile_pool(name="srpsum", bufs=1, space="PSUM") as srpsum,
            ):
                for h in range(HL):
                    # pass 1: PV over qb 0,1 (A-half); emit B-half scores of h
                    pso0 = pvpsum.tile([HD + 1, 512], F32, tag="pv", name=f"pva{h}")
                    pso1 = pvpsum.tile([HD + 1, 512], F32, tag="pv", name=f"pvb{h}")
                    for kc in range(NKC):
                        scores_half(h, kc, 1)
                        pv_step(pso0, h, 0, kc)
                        pv_step(pso1, h, 1, kc)
                    normalize(pso0, h, 0)
                    normalize(pso1, h, 1)
                    # pass 2: PV over qb 2,3 (B-half); emit A-half scores of
                    # h+1, or the first 8 outproj tiles when h == 3
                    pso2 = pvpsum.tile([HD + 1, 512], F32, tag="pv", name=f"pvc{h}")
                    pso3 = pvpsum.tile([HD + 1, 512], F32, tag="pv", name=f"pvd{h}")
                    for kc in range(NKC):
                        if h < HL - 1:
                            scores_half(h + 1, kc, 0)
                        elif kc % 2 == 0:
                            outproj(kc // 2)
                        pv_step(pso2, h, 2, kc)
                        pv_step(pso3, h, 3, kc)
                    normalize(pso2, h, 2)
                    normalize(pso3, h, 3)
                for qt in range(8, 16):
                    outproj(qt)

    nc.finalize()
    return nc


_NC = None


def _get_nc():
    global _NC
    if _NC is None:
        _NC = build_nc()
    return _NC


def _tile_bias(b):
    # [128,4]: row r, col fc -> b[fc*64 + (r % 64)]; correct whether the hw
    # indexes the scalar operand by input or output lane
    return np.ascontiguousarray(
        np.tile(np.asarray(b, np.float32).reshape(4, HD).T, (2, 1))
    )


def make_in_maps(Q, K, mask, Wq, bq, Wk, bk, Wv, bv, Wo, bo):
    Q = np.asarray(Q, np.float32)
    K = np.asarray(K, np.float32)
    mask = np.asarray(mask)
    Wq = np.asarray(Wq, np.float32)
    Wk = np.asarray(Wk, np.float32)
    Wv = np.asarray(Wv, np.float32)
    Wo = np.asarray(Wo, np.float32)
    qt = [np.ascontiguousarray(Q[b].T).astype(NP_FP8) for b in range(B)]
    ktf = [np.ascontiguousarray(K[b].T) for b in range(B)]
    kt8 = [k.astype(NP_FP8) for k in ktf]
    ktb = [k.astype(NP_BF16) for k in ktf]
    mt = [np.ascontiguousarray(mask[b].T).astype(NP_BF16) for b in range(B)]
    in_maps = []
    for c in range(8):
        b, hg = divmod(c, 4)
        cols = slice(hg * DL, (hg + 1) * DL)
        in_maps.append(
            {
                "qt": qt[b],
                "kt": kt8[b],
                "ktb": ktb[b],
                "mt": mt[b],
                "wqt": np.ascontiguousarray(Wq[cols, :].T * WS).astype(NP_FP8),
                "wkt": np.ascontiguousarray(Wk[cols, :].T * WS).astype(NP_FP8),
                "wvt": np.ascontiguousarray(Wv[cols, :].T).astype(NP_BF16),
                "wot": np.ascontiguousarray(Wo[:, cols].T).astype(NP_BF16),
                "bqt": _tile_bias(np.asarray(bq, np.float32)[cols]),
                "bkt": _tile_bias(np.asarray(bk, np.float32)[cols]),
                "bv": np.ascontiguousarray(np.asarray(bv, np.float32)[cols]),
                "ones": np.ones(HD, np.float32),
            }
        )
    return in_maps


def assemble(results, bo):
    O = np.zeros((B, N, D), np.float32)
    for c in range(8):
        b = c // 4
        O[b] += results[c]["out"]
    O += np.asarray(bo, np.float32)[None, None, :]
    return O


def kernel(Q, K, mask, Wq, bq, Wk, bk, Wv, bv, Wo, bo):
    nc = _get_nc()
    in_maps = make_in_maps(Q, K, mask, Wq, bq, Wk, bk, Wv, bv, Wo, bo)
    res = run_bass_kernel_spmd(nc, in_maps, core_ids=list(range(8)))
    return assemble(res.results, bo)


# revision 22
# speedup vs baseline: 1.0994x; 1.0994x over previous
"""Trainium2 Bass kernel for nn_MHA_36584531427723.

Sharding: 8 cores = 2 batches x 4 head-groups (4 heads of 64 dims each per
core). Each core computes its batch's Q/K/V projections restricted to its
head-group's 256 output features, attention for its 4 heads, and a partial
output projection (its 256 rows of Wo^T). The host sums the 4 partials per
batch and adds bo.

v3 design notes (engine balance + pipeline; cost-model driven):
  - Q/K projections run as fp8e4m3 DoubleRow matmuls (contraction 256/instr,
    0.5 cyc/row, psum output must start at partition 0 on hw): weights are
    host-prescaled by 64 (fp8 min-normal is 2^-6, raw weights sigma 0.02);
    the evict rescales by 1/64 and adds the bias. The V projection stays bf16:
    measured offline, fp8 V alone costs 1.8e-2 relative error (V errors pass
    straight to the output) while fp8 Q+K costs only 3.9e-3.
  - Activation runs ONLY the 256 exp instructions [128,1024] (~133us); all
    psum evicts go to DVE (GpSimd cannot touch PSUM on real hw).
  - Softmax denominator comes from a ones-column appended to V (psum row 64):
    reciprocal (DVE, crosses partitions 64->0), broadcast via a tiny f32r
    matmul into psum, then o = srp * o_raw writes oT at the head's partition
    base (DVE maps in/out partition ranges by index; verified on hw).
  - PV runs in two passes (qb 0,1 then qb 2,3) with kc as the OUTER loop and
    two live psum accumulators per pass. Each ex tile is [128,1024] (one
    q-half): the A-half ring is released during pass 1 and the B-half ring
    during pass 2, so head h+1's scores/exp stream while head h's PV runs
    with only ~32 ex tiles of SBUF.
  - Emission interleaving: pass1(h) emits B-half scores(h); pass2(h) emits
    A-half scores(h+1) (or the first 8 output-projection tiles when h=3).
    Phase A emits A-half scores of head 0 as K/V panels complete.
  - Mask multiply is split ~7/16 to GpSimd to balance DVE/Pool near 120us.
  - Biases for the Q/K evicts are host-tiled to [128,4] so the value is
    correct whether the scalar operand indexes by input or output lane.
  - PSUM: scores 2x[128,1024](4) + PV 2x[65,512](2) + srp(1) + outproj(1) = 8.
  - softmax max-subtraction and the +eps are dropped (|E|<~1 so exp is safe;
    relative effect ~1e-11).
"""

import numpy as np
import ml_dtypes

import concourse.bacc as bacc
import concourse.bass as bass  # noqa: F401
import concourse.mybir as mybir
import concourse.tile as tile
from concourse.bass_utils import run_bass_kernel_spmd

B, N, D = 2, 2048, 1024
H = 16
HD = 64
HL = 4  # heads per core
DL = HL * HD  # 256 local features
P = 128
KO = D // P  # 8 contraction chunks of the input feature dim
KO2 = KO // 2  # 4 DoubleRow chunk-pairs
NKC = N // P  # 16 k-token chunks
NPAN = 4
PANW = N // NPAN  # 512-wide token panels in the projection phase
SCALE = 1.0 / 32.0  # 1/sqrt(DIM_V)
WS = 64.0  # host-side weight prescale for fp8
INV_WS = 1.0 / WS

F32 = mybir.dt.float32
F32R = mybir.dt.float32r
BF16 = mybir.dt.bfloat16
FP8 = mybir.dt.float8e4
AF = mybir.ActivationFunctionType
ALU = mybir.AluOpType
DR = mybir.MatmulPerfMode.DoubleRow

NP_FP8 = ml_dtypes.float8_e4m3
NP_BF16 = ml_dtypes.bfloat16


def build_nc():
    nc = bacc.Bacc(None, target_bir_lowering=False)
    QT = nc.dram_tensor("qt", (D, N), FP8, kind="ExternalInput")
    KT = nc.dram_tensor("kt", (D, N), FP8, kind="ExternalInput")
    KTB = nc.dram_tensor("ktb", (D, N), BF16, kind="ExternalInput")
    MT = nc.dram_tensor("mt", (N, N), BF16, kind="ExternalInput")
    WQT = nc.dram_tensor("wqt", (D, DL), FP8, kind="ExternalInput")
    WKT = nc.dram_tensor("wkt", (D, DL), FP8, kind="ExternalInput")
    WVT = nc.dram_tensor("wvt", (D, DL), BF16, kind="ExternalInput")
    WOT = nc.dram_tensor("wot", (DL, D), BF16, kind="ExternalInput")
    BQT = nc.dram_tensor("bqt", (P, 4), F32, kind="ExternalInput")
    BKT = nc.dram_tensor("bkt", (P, 4), F32, kind="ExternalInput")
    BV = nc.dram_tensor("bv", (DL,), F32, kind="ExternalInput")
    ONES = nc.dram_tensor("ones", (HD,), F32R, kind="ExternalInput")
    OUT = nc.dram_tensor("out", (N, D), BF16, kind="ExternalOutput")

    qt_r = QT[:].rearrange("(ko p) q -> p ko q", p=P)
    kt_r = KT[:].rearrange("(ko p) q -> p ko q", p=P)
    ktb_r = KTB[:].rearrange("(ko p) q -> p ko q", p=P)
    mt_r = MT[:].rearrange("(kc p) q -> p kc q", p=P)

    with tile.TileContext(nc) as tc:
        with (
            tc.tile_pool(name="persist", bufs=1) as persist,
            tc.tile_pool(name="expool", bufs=15) as expool,
            tc.tile_pool(name="sipool", bufs=2) as sipool,
            tc.tile_pool(name="osb", bufs=3) as osb,
            tc.tile_pool(name="spsum", bufs=2, space="PSUM") as spsum,
        ):
            # --- persistent tiles ---
            mT = persist.tile([P, NKC, N], BF16)  # 64KB/part
            qT = persist.tile([P, 2, N], BF16, tag="qT")
            kT = persist.tile([P, 2, N], BF16, tag="kT")
            oT = persist.tile([P, 2, N], BF16, tag="oT")
            v_sb = persist.tile([P, NKC, HL, HD + 1], BF16, tag="v")
            ones_sb = persist.tile([1, HD], F32R, tag="ones")
            bqt_sb = persist.tile([P, 4], F32, tag="bqt")
            bkt_sb = persist.tile([P, 4], F32, tag="bkt")
            bv_rep = persist.tile([P, DL], F32, tag="bv")
            wo_sb = persist.tile([P, 2, D], BF16, tag="wo")
            wq_sb = persist.tile([P, KO, DL], FP8, tag="wq")
            wk_sb = persist.tile([P, KO, DL], FP8, tag="wk")
            wv_sb = persist.tile([P, KO, DL], BF16, tag="wv")

            nc.vector.memset(v_sb[:, :, :, HD : HD + 1], 1.0)
            nc.sync.dma_start(out=ones_sb[:], in_=ONES[:][None])
            nc.sync.dma_start(out=bqt_sb[:], in_=BQT[:])
            nc.sync.dma_start(out=bkt_sb[:], in_=BKT[:])
            nc.sync.dma_start(out=bv_rep[:], in_=BV[:][None].to_broadcast((P, DL)))
            for w_sb, W in ((wq_sb, WQT), (wk_sb, WKT), (wv_sb, WVT)):
                nc.sync.dma_start(
                    out=w_sb[:], in_=W[:].rearrange("(ko p) m -> p ko m", p=P)
                )
            for cc in range(2):
                nc.sync.dma_start(
                    out=wo_sb[:, cc, :],
                    in_=WOT[:].rearrange("(cc p) n -> p cc n", p=P)[:, cc, :],
                )

            exA = [None] * NKC  # current A-half (q 0:1024) ex tiles, by kc
            exB = [None] * NKC  # current B-half (q 1024:2048)

            def scores_half(h, kc, half):
                """Scores+exp+mask for one [128 k, 1024 q] tile."""
                dc, po = h // 2, (h % 2) * HD
                ex = expool.tile(
                    [P, 1024],
                    BF16,
                    tag=("exA" if half == 0 else "exB"),
                    name=f"ex{h}_{kc}_{half}",
                )
                (exA if half == 0 else exB)[kc] = ex
                ps = spsum.tile([P, 1024], F32, tag="es", name=f"es{h}_{kc}_{half}")
                for j in range(2):
                    q0 = half * 1024 + j * 512
                    nc.tensor.matmul(
                        ps[:, j * 512 : (j + 1) * 512],
                        lhsT=kT[po : po + HD, dc, kc * P : (kc + 1) * P],
                        rhs=qT[po : po + HD, dc, q0 : q0 + 512],
                        start=True,
                        stop=True,
                    )
                nc.scalar.activation(out=ex[:], in_=ps[:], func=AF.Exp, scale=SCALE)
                eng = nc.gpsimd if (2 * kc + half) % 16 < 7 else nc.vector
                eng.tensor_mul(
                    out=ex[:],
                    in0=ex[:],
                    in1=mT[:, kc, half * 1024 : half * 1024 + 1024],
                )

            def pv_step(pso, h, qb, kc):
                exh = (exA if qb < 2 else exB)[kc]
                nc.tensor.matmul(
                    pso[:],
                    lhsT=v_sb[:, kc, h, :],
                    rhs=exh[:, (qb % 2) * 512 : (qb % 2) * 512 + 512],
                    start=(kc == 0),
                    stop=(kc == NKC - 1),
                )

            def normalize(pso, h, qb):
                dc, po = h // 2, (h % 2) * HD
                s_inv = sipool.tile([1, 512], F32R, tag="si", name=f"si{h}_{qb}")
                with nc.allow_low_precision(reason="f32r is bit-identical to f32"):
                    nc.vector.reciprocal(out=s_inv[:], in_=pso[HD : HD + 1, :])
                o_raw = sipool.tile([HD, 512], BF16, tag="oraw", name=f"or{h}_{qb}")
                nc.vector.tensor_copy(out=o_raw[:], in_=pso[0:HD, :])
                srp = srpsum.tile([HD, 512], F32, tag="srp", name=f"srp{h}_{qb}")
                nc.tensor.matmul(
                    srp[:], lhsT=ones_sb[:], rhs=s_inv[:], start=True, stop=True
                )
                q0 = qb * 512
                nc.vector.tensor_mul(
                    out=oT[po : po + HD, dc, q0 : q0 + 512], in0=srp[:], in1=o_raw[:]
                )

            def outproj(qt):
                # borrows a scores-ring psum tile (free of scores work by the
                # time outproj runs); single evict + single store per qt
                ps = spsum.tile([P, 1024], F32, tag="es", name=f"co{qt}")
                for nh in range(2):
                    for cc in range(2):
                        nc.tensor.matmul(
                            ps[:, nh * 512 : (nh + 1) * 512],
                            lhsT=oT[:, cc, qt * P : (qt + 1) * P],
                            rhs=wo_sb[:, cc, nh * 512 : (nh + 1) * 512],
                            start=(cc == 0),
                            stop=(cc == 1),
                        )
                o_sb = osb.tile([P, D], BF16, tag="osb", name=f"osb{qt}")
                nc.vector.tensor_copy(out=o_sb[:], in_=ps[:])
                q_eng = nc.scalar if qt % 2 == 0 else nc.sync
                q_eng.dma_start(out=OUT[qt * P : (qt + 1) * P, :], in_=o_sb[:])

            # ---------------- Phase A: projections (+ h0 A-half scores) -----
            with (
                tc.tile_pool(name="panpool", bufs=2) as panpool,
                tc.tile_pool(name="panbpool", bufs=1) as panbpool,
                tc.tile_pool(name="pjpsum", bufs=2, space="PSUM") as pjpsum,
                tc.tile_pool(name="vpsum", bufs=2, space="PSUM") as vpsum,
            ):

                def q_panel(pan):
                    qs = slice(pan * PANW, (pan + 1) * PANW)
                    qt_pan = panpool.tile([P, KO, PANW], FP8, tag="pan", name=f"qp{pan}")
                    nc.sync.dma_start(out=qt_pan[:], in_=qt_r[:, :, qs])
                    for fc in range(4):
                        dc, po = fc // 2, (fc % 2) * HD
                        ps = pjpsum.tile([HD, PANW], F32, tag="pj", name=f"pjq{pan}_{fc}")
                        for ko2 in range(KO2):
                            nc.tensor.matmul(
                                ps[:],
                                lhsT=wq_sb[
                                    :, 2 * ko2 : 2 * ko2 + 2, fc * HD : (fc + 1) * HD
                                ],
                                rhs=qt_pan[:, 2 * ko2 : 2 * ko2 + 2, :],
                                start=(ko2 == 0),
                                stop=(ko2 == KO2 - 1),
                                perf_mode=DR,
                            )
                        nc.vector.tensor_scalar(
                            out=qT[po : po + HD, dc, qs],
                            in0=ps[:],
                            scalar1=INV_WS,
                            scalar2=bqt_sb[po : po + HD, fc : fc + 1],
                            op0=ALU.mult,
                            op1=ALU.add,
                        )

                def kv_panel(pan, b_early=()):
                    qs = slice(pan * PANW, (pan + 1) * PANW)
                    nc.scalar.dma_start(
                        out=mT[:, pan * 4 : pan * 4 + 4, :],
                        in_=mt_r[:, pan * 4 : pan * 4 + 4, :],
                    )
                    kt_pan = panpool.tile([P, KO, PANW], FP8, tag="pan", name=f"kp{pan}")
                    ktb_pan = panbpool.tile(
                        [P, KO, PANW], BF16, tag="panb", name=f"kbp{pan}"
                    )
                    nc.sync.dma_start(out=kt_pan[:], in_=kt_r[:, :, qs])
                    nc.sync.dma_start(out=ktb_pan[:], in_=ktb_r[:, :, qs])
                    for fc in range(4):
                        dc, po = fc // 2, (fc % 2) * HD
                        ps = pjpsum.tile([HD, PANW], F32, tag="pj", name=f"pjk{pan}_{fc}")
                        for ko2 in range(KO2):
                            nc.tensor.matmul(
                                ps[:],
                                lhsT=wk_sb[
                                    :, 2 * ko2 : 2 * ko2 + 2, fc * HD : (fc + 1) * HD
                                ],
                                rhs=kt_pan[:, 2 * ko2 : 2 * ko2 + 2, :],
                                start=(ko2 == 0),
                                stop=(ko2 == KO2 - 1),
                                perf_mode=DR,
                            )
                        nc.vector.tensor_scalar(
                            out=kT[po : po + HD, dc, qs],
                            in0=ps[:],
                            scalar1=INV_WS,
                            scalar2=bkt_sb[po : po + HD, fc : fc + 1],
                            op0=ALU.mult,
                            op1=ALU.add,
                        )
                    # V projection in bf16 (fp8 V costs 1.8e-2 output error)
                    for tc128 in range(PANW // P):
                        kc = pan * 4 + tc128
                        psv = vpsum.tile(
                            [P, DL], F32, tag="pv8", name=f"pv8{pan}_{tc128}"
                        )
                        for ko in range(KO):
                            nc.tensor.matmul(
                                psv[:],
                                lhsT=ktb_pan[:, ko, tc128 * P : (tc128 + 1) * P],
                                rhs=wv_sb[:, ko, :],
                                start=(ko == 0),
                                stop=(ko == KO - 1),
                            )
                        nc.vector.tensor_add(
                            out=v_sb[:, kc, :, 0:HD],
                            in0=psv[:].rearrange("p (h d) -> p h d", h=HL),
                            in1=bv_rep[:].rearrange("p (h d) -> p h d", h=HL),
                        )
                    # head-0 A-half scores (need qT q 0:1024 = Q panels 0,1)
                    for kc in range(pan * 4, pan * 4 + 4):
                        scores_half(0, kc, 0)
                    # B-half scores for earlier chunks once qT is complete
                    for kc in b_early:
                        scores_half(0, kc, 1)

                # A-half scores only need Q panels 0,1 -- start attention as
                # early as possible and fill the rest of Q between K/V panels
                q_panel(0)
                q_panel(1)
                kv_panel(0)
                q_panel(2)
                kv_panel(1)
                q_panel(3)
                kv_panel(2, b_early=range(0, 4))
                kv_panel(3, b_early=range(4, 8))

            # ---------------- Phase B+C: attention pipeline ----------------
            with (
                tc.tile_pool(name="pvpsum", bufs=3, space="PSUM") as pvpsum,
                tc.tile_pool(name="srpsum", bufs=1, space="PSUM") as srpsum,
            ):
                for h in range(HL):
                    # pass 1: PV over qb 0,1 (A-half); emit B-half scores of h
                    pso0 = pvpsum.tile([HD + 1, 512], F32, tag="pv", name=f"pva{h}")
                    pso1 = pvpsum.tile([HD + 1, 512], F32, tag="pv", name=f"pvb{h}")
                    for kc in range(NKC):
                        if h > 0 or kc >= 8:
                            scores_half(h, kc, 1)
                        pv_step(pso0, h, 0, kc)
                        pv_step(pso1, h, 1, kc)
                    normalize(pso0, h, 0)
                    normalize(pso1, h, 1)
                    # pass 2: PV over qb 2,3 (B-half); emit A-half scores of
                    # h+1, or the first 8 outproj tiles when h == 3
                    pso2 = pvpsum.tile([HD + 1, 512], F32, tag="pv", name=f"pvc{h}")
                    pso3 = pvpsum.tile([HD + 1, 512], F32, tag="pv", name=f"pvd{h}")
                    for kc in range(NKC):
                        if h < HL - 1:
                            scores_half(h + 1, kc, 0)
                        elif kc % 2 == 0:
                            outproj(kc // 2)
                        pv_step(pso2, h, 2, kc)
                        pv_step(pso3, h, 3, kc)
                    normalize(pso2, h, 2)
                    normalize(pso3, h, 3)
                for qt in range(8, 16):
                    outproj(qt)

    nc.finalize()
    return nc


_NC = None


def _get_nc():
    global _NC
    if _NC is None:
        _NC = build_nc()
    return _NC


def _tile_bias(b):
    # [128,4]: row r, col fc -> b[fc*64 + (r % 64)]; correct whether the hw
    # indexes the scalar operand by input or output lane
    return np.ascontiguousarray(
        np.tile(np.asarray(b, np.float32).reshape(4, HD).T, (2, 1))
    )


def make_in_maps(Q, K, mask, Wq, bq, Wk, bk, Wv, bv, Wo, bo):
    Q = np.asarray(Q, np.float32)
    K = np.asarray(K, np.float32)
    mask = np.asarray(mask)
    Wq = np.asarray(Wq, np.float32)
    Wk = np.asarray(Wk, np.float32)
    Wv = np.asarray(Wv, np.float32)
    Wo = np.asarray(Wo, np.float32)
    qt = [np.ascontiguousarray(Q[b].T).astype(NP_FP8) for b in range(B)]
    ktf = [np.ascontiguousarray(K[b].T) for b in range(B)]
    kt8 = [k.astype(NP_FP8) for k in ktf]
    ktb = [k.astype(NP_BF16) for k in ktf]
    mt = [np.ascontiguousarray(mask[b].T).astype(NP_BF16) for b in range(B)]
    in_maps = []
    for c in range(8):
        b, hg = divmod(c, 4)
        cols = slice(hg * DL, (hg + 1) * DL)
        in_maps.append(
            {
                "qt": qt[b],
                "kt": kt8[b],
                "ktb": ktb[b],
                "mt": mt[b],
                "wqt": np.ascontiguousarray(Wq[cols, :].T * WS).astype(NP_FP8),
                "wkt": np.ascontiguousarray(Wk[cols, :].T * WS).astype(NP_FP8),
                "wvt": np.ascontiguousarray(Wv[cols, :].T).astype(NP_BF16),
                "wot": np.ascontiguousarray(Wo[:, cols].T).astype(NP_BF16),
                "bqt": _tile_bias(np.asarray(bq, np.float32)[cols]),
                "bkt": _tile_bias(np.asarray(bk, np.float32)[cols]),
                "bv": np.ascontiguousarray(np.asarray(bv, np.float32)[cols]),
                "ones": np.ones(HD, np.float32),
            }
        )
    return in_maps


def assemble(results, bo):
    O = np.zeros((B, N, D), np.float32)
    for c in range(8):
        b = c // 4
        O[b] += results[c]["out"].astype(np.float32)
    O += np.asarray(bo, np.float32)[None, None, :]
    return O


def kernel(Q, K, mask, Wq, bq, Wk, bk, Wv, bv, Wo, bo):
    nc = _get_nc()
    in_maps = make_in_maps(Q, K, mask, Wq, bq, Wk, bk, Wv, bv, Wo, bo)
    res = run_bass_kernel_spmd(nc, in_maps, core_ids=list(range(8)))
    return assemble(res.results, bo)
